# revision 1
# baseline (speedup 1.0000x reference)
"""Trainium2 Bass kernel for CustomRoPEAttention (B=2, S=2048, H=16, Dh=128).

Sharding: 8 cores = 2 batches x 4 head-groups (4 heads/core), tensor-parallel
over heads + data-parallel over batch. Each core computes QKV projection for
its heads (f32r matmuls), RoPE, causal softmax attention, and a partial
(transposed) output projection. Host sums the 4 partials per batch + bias.

Self-contained: hardcodes shapes from the problem spec.
"""
import math
from contextlib import ExitStack

import numpy as np

import concourse.mybir as mybir
import concourse.tile as tile
from concourse import bacc
from concourse.bass_utils import run_bass_kernel_spmd
from concourse.masks import make_identity

S = 2048            # sequence
D = 2048            # hidden
NH = 16             # total heads
DH = 128            # head dim
HG = 4              # heads per core
GQ = HG * DH        # 512: per-core q/k/v feature width
B = 2
NCORES = 8
ROPE_THETA = 10000.0
SCALE = 1.0 / math.sqrt(DH)
NEG = -1.0e9
SLAB = 256          # phase-1 sequence slab width
F32 = mybir.dt.float32
F32R = mybir.dt.float32r
MULT = mybir.AluOpType.mult
ADD = mybir.AluOpType.add


def build_nc():
    nc = bacc.Bacc(None, target_bir_lowering=False)
    xt = nc.dram_tensor("xt", [16, 128, S], F32R, kind="ExternalInput")       # x^T tiles [kc,p,s]
    wqk = nc.dram_tensor("wqk", [16, 128, 2 * GQ], F32R, kind="ExternalInput")
    wv = nc.dram_tensor("wv", [16, 128, GQ], F32R, kind="ExternalInput")
    wo = nc.dram_tensor("wo", [4, 128, D], F32R, kind="ExternalInput")        # Wo rows tiles
    bqk = nc.dram_tensor("bqk", [1, 2 * GQ], F32R, kind="ExternalInput")
    bv = nc.dram_tensor("bv", [1, GQ], F32R, kind="ExternalInput")
    cost = nc.dram_tensor("cost", [128, S], F32, kind="ExternalInput")        # cos^T
    sinrt = nc.dram_tensor("sinrt", [128, S], F32, kind="ExternalInput")      # sin^T with rot sign
    maskd = nc.dram_tensor("maskd", [128, 128], F32, kind="ExternalInput")    # diag causal add-mask
    outt = nc.dram_tensor("outt", [16, 128, S], F32, kind="ExternalOutput")   # partial^T tiles
    qks = nc.dram_tensor("qks", [2 * HG, 128, S], F32R)                       # spill: q then k head tiles
    vsp = nc.dram_tensor("vsp", [16, 128, GQ], F32R)                          # spill: V natural tiles

    with tile.TileContext(nc) as tc, ExitStack() as top:
        g = top.enter_context(tc.tile_pool(name="glob", bufs=1))
        tcos = g.tile([128, S], F32)
        nc.sync.dma_start(out=tcos, in_=cost[:])
        tsin = g.tile([128, S], F32)
        nc.sync.dma_start(out=tsin, in_=sinrt[:])
        tmask = g.tile([128, 128], F32)
        nc.sync.dma_start(out=tmask, in_=maskd[:])
        ident_f = g.tile([128, 128], F32)
        make_identity(nc, ident_f[:])
        ident = g.tile([128, 128], F32R)
        nc.vector.tensor_copy(out=ident[:], in_=ident_f[:])
        ones_f = g.tile([1, SLAB], F32)
        nc.vector.memset(ones_f[:], 1.0)
        ones = g.tile([1, SLAB], F32R)
        nc.vector.tensor_copy(out=ones[:], in_=ones_f[:])
        tbqk = g.tile([1, 2 * GQ], F32R)
        nc.sync.dma_start(out=tbqk, in_=bqk[:])
        tbv = g.tile([1, GQ], F32R)
        nc.sync.dma_start(out=tbv, in_=bv[:])
        # C^T [ (head,dh), S ] persists from phase 2 into phase 3
        ctp = top.enter_context(tc.tile_pool(name="ctp", bufs=1))
        ct_sb = ctp.tile([128, HG, S], F32R)

        # ---------------- Phase 1: QKV^T projection + RoPE + spill ----------------
        with tc.tile_pool(name="p1w", bufs=1) as p1w, \
             tc.tile_pool(name="p1x", bufs=2) as p1x, \
             tc.tile_pool(name="p1s", bufs=3) as p1s, \
             tc.tile_pool(name="p1ps", bufs=2, space="PSUM") as p1ps, \
             tc.tile_pool(name="p1vps", bufs=2, space="PSUM") as p1vps:
            twqk = p1w.tile([128, 16, 2 * GQ], F32R)
            nc.sync.dma_start(out=twqk, in_=wqk.rearrange("kc p f -> p kc f"))
            twv = p1w.tile([128, 16, GQ], F32R)
            nc.sync.dma_start(out=twv, in_=wv.rearrange("kc p f -> p kc f"))
            for ns in range(S // SLAB):
                sl = slice(ns * SLAB, (ns + 1) * SLAB)
                xs = p1x.tile([128, 16, SLAB], F32R, tag="xs")
                nc.sync.dma_start(out=xs, in_=xt[:, :, sl].rearrange("kc p s -> p kc s"))
                # Q^T and K^T head tiles (mt 0..3 = q heads, 4..7 = k heads)
                for mt in range(2 * HG):
                    pqk = p1ps.tile([128, SLAB], F32, tag="qkps")
                    nc.tensor.matmul(pqk[:], tbqk[:, mt * 128:(mt + 1) * 128],
                                     ones[:], start=True, stop=False)
                    for kc in range(16):
                        nc.tensor.matmul(pqk[:], twqk[:, kc, mt * 128:(mt + 1) * 128],
                                         xs[:, kc, :], start=False, stop=(kc == 15))
                    qraw = p1s.tile([128, SLAB], F32, tag="qraw")
                    nc.scalar.copy(out=qraw[:], in_=pqk[:])
                    qsw = p1s.tile([128, SLAB], F32, tag="qsw")
                    nc.sync.dma_start(out=qsw[0:64, :], in_=qraw[64:128, :])
                    nc.sync.dma_start(out=qsw[64:128, :], in_=qraw[0:64, :])
                    m1 = p1s.tile([128, SLAB], F32R, tag="m1")
                    nc.vector.tensor_tensor(out=m1[:], in0=pqk[:], in1=tcos[:, sl], op=MULT)
                    m2 = p1s.tile([128, SLAB], F32R, tag="m2")
                    nc.vector.tensor_tensor(out=m2[:], in0=qsw[:], in1=tsin[:, sl], op=MULT)
                    qf = p1s.tile([128, SLAB], F32R, tag="qf")
                    nc.vector.tensor_tensor(out=qf[:], in0=m1[:], in1=m2[:], op=ADD)
                    nc.sync.dma_start(out=qks[mt, :, sl], in_=qf[:])
                # V natural tiles for this slab
                for st in range(SLAB // 128):
                    pv = p1vps.tile([128, GQ], F32, tag="vps")
                    nc.tensor.matmul(pv[:], ones[:, 0:128], tbv[:], start=True, stop=False)
                    s0 = st * 128
                    for kc in range(16):
                        nc.tensor.matmul(pv[:], xs[:, kc, s0:s0 + 128],
                                         twv[:, kc, :], start=False, stop=(kc == 15))
                    vsb = p1s.tile([128, GQ], F32R, tag="vsb")
                    nc.scalar.copy(out=vsb[:], in_=pv[:])
                    nc.sync.dma_start(out=vsp[ns * (SLAB // 128) + st], in_=vsb[:])

        # ---------------- Phase 2: attention per head ----------------
        with tc.tile_pool(name="p2h", bufs=2) as p2h, \
             tc.tile_pool(name="p2a", bufs=3) as p2a, \
             tc.tile_pool(name="p2t", bufs=2) as p2t, \
             tc.tile_pool(name="p2sp", bufs=3, space="PSUM") as p2sp, \
             tc.tile_pool(name="p2tp", bufs=2, space="PSUM") as p2tp, \
             tc.tile_pool(name="p2cp", bufs=2, space="PSUM") as p2cp:
            for h in range(HG):
                qh = p2h.tile([128, S], F32R, tag="qh")
                nc.sync.dma_start(out=qh, in_=qks[h])
                kh = p2h.tile([128, S], F32R, tag="kh")
                nc.sync.dma_start(out=kh, in_=qks[HG + h])
                vh = p2h.tile([128, 16, 128], F32R, tag="vh")
                nc.sync.dma_start(out=vh,
                                  in_=vsp[:, :, h * 128:(h + 1) * 128].rearrange("t p f -> p t f"))
                for j in range(8):
                    at_sb = p2t.tile([128, 16, 256], F32R, tag="atsb")
                    for ii, i in enumerate((2 * j, 2 * j + 1)):
                        ski = (i + 1) * 128
                        nchunk = (ski + 511) // 512
                        ai = p2a.tile([128, S], F32R, tag="ai")
                        hs = p2a.tile([128, 4], F32, tag="hs")
                        for cc in range(nchunk):
                            off = cc * 512
                            w = min(512, ski - off)
                            sp = p2sp.tile([128, 512], F32, tag="sp")
                            nc.tensor.matmul(sp[:, 0:w], qh[:, i * 128:(i + 1) * 128],
                                             kh[:, off:off + w], start=True, stop=True)
                            if off <= i * 128 < off + w:  # diagonal block lives here
                                dd = i * 128 - off
                                nc.vector.tensor_tensor(out=sp[:, dd:dd + 128],
                                                        in0=sp[:, dd:dd + 128],
                                                        in1=tmask[:], op=ADD)
                            nc.scalar.activation(out=ai[:, off:off + w], in_=sp[:, 0:w],
                                                 func=mybir.ActivationFunctionType.Exp,
                                                 scale=SCALE, accum_out=hs[:, cc:cc + 1])
                        for cc in range(1, nchunk):
                            nc.vector.tensor_tensor(out=hs[:, 0:1], in0=hs[:, 0:1],
                                                    in1=hs[:, cc:cc + 1], op=ADD)
                        rec = p2a.tile([128, 1], F32, tag="rec")
                        nc.vector.reciprocal(out=rec[:], in_=hs[:, 0:1])
                        nc.vector.tensor_tensor(out=ai[:, 0:ski], in0=ai[:, 0:ski],
                                                in1=rec[:].broadcast_to((128, ski)), op=MULT)
                        for ks in range(i + 1):
                            atp = p2tp.tile([128, 128], F32R, tag="atp")
                            nc.tensor.transpose(atp[:], ai[:, ks * 128:(ks + 1) * 128], ident[:])
                            nc.scalar.copy(out=at_sb[:, ks, ii * 128:(ii + 1) * 128], in_=atp[:])
                    ct = p2cp.tile([128, 256], F32, tag="ct")
                    for ks in range(2 * j + 1):
                        nc.tensor.matmul(ct[:], vh[:, ks, :], at_sb[:, ks, :],
                                         start=(ks == 0), stop=False)
                    nc.tensor.matmul(ct[:, 128:256], vh[:, 2 * j + 1, :],
                                     at_sb[:, 2 * j + 1, 128:256], start=False, stop=True)
                    nc.scalar.copy(out=ct_sb[:, h, j * 256:(j + 1) * 256], in_=ct[:])

        # ---------------- Phase 3: output projection (transposed partial) ----------------
        with tc.tile_pool(name="p3w", bufs=1) as p3w, \
             tc.tile_pool(name="p3s", bufs=4) as p3s, \
             tc.tile_pool(name="p3ps", bufs=4, space="PSUM") as p3ps:
            two = p3w.tile([128, 4, D], F32R)
            nc.sync.dma_start(out=two, in_=wo.rearrange("kc p f -> p kc f"))
            for mt in range(16):
                for ncc in range(4):
                    op = p3ps.tile([128, 512], F32, tag="op")
                    for kh in range(4):
                        nc.tensor.matmul(op[:], two[:, kh, mt * 128:(mt + 1) * 128],
                                         ct_sb[:, kh, ncc * 512:(ncc + 1) * 512],
                                         start=(kh == 0), stop=(kh == 3))
                    ob = p3s.tile([128, 512], F32, tag="ob")
                    nc.vector.tensor_copy(out=ob[:], in_=op[:])
                    nc.sync.dma_start(out=outt[mt, :, ncc * 512:(ncc + 1) * 512], in_=ob[:])
    nc.finalize()
    return nc


_NC_CACHE = []


def _get_nc():
    if not _NC_CACHE:
        _NC_CACHE.append(build_nc())
    return _NC_CACHE[0]


def _rope_tables(position_ids_b):
    pos = position_ids_b.astype(np.float32)
    inv_freq = (1.0 / (ROPE_THETA ** (np.arange(0, DH, 2, dtype=np.float32) / np.float32(DH))))
    ang = pos[:, None] * inv_freq[None, :]          # [S, 64]
    emb = np.concatenate([ang, ang], axis=-1)       # [S, 128]
    cosT = np.ascontiguousarray(np.cos(emb).T)      # [128, S]
    sinT = np.sin(emb).T
    sin_rot = np.concatenate([-sinT[0:64], sinT[64:128]], axis=0)
    return cosT.astype(np.float32), np.ascontiguousarray(sin_rot).astype(np.float32)


def kernel(hidden_states, position_ids, Wqkv, bqkv, Wo, bo):
    hidden_states = np.asarray(hidden_states, dtype=np.float32)
    Wqkv = np.asarray(Wqkv, dtype=np.float32)
    bqkv = np.asarray(bqkv, dtype=np.float32)
    Wo = np.asarray(Wo, dtype=np.float32)
    bo = np.asarray(bo, dtype=np.float32)

    mask = np.triu(np.full((128, 128), NEG, dtype=np.float32), k=1)
    tabs = [_rope_tables(np.asarray(position_ids)[b]) for b in range(B)]
    xts = [np.ascontiguousarray(hidden_states[b].T).reshape(16, 128, S) for b in range(B)]

    in_maps = []
    for c in range(NCORES):
        b, hg = divmod(c, HG)
        qcols = slice(hg * GQ, (hg + 1) * GQ)
        kcols = slice(D + hg * GQ, D + (hg + 1) * GQ)
        vcols = slice(2 * D + hg * GQ, 2 * D + (hg + 1) * GQ)
        wqk_c = np.ascontiguousarray(
            np.concatenate([Wqkv[:, qcols], Wqkv[:, kcols]], axis=1)).reshape(16, 128, 2 * GQ)
        wv_c = np.ascontiguousarray(Wqkv[:, vcols]).reshape(16, 128, GQ)
        wo_c = np.ascontiguousarray(Wo[hg * GQ:(hg + 1) * GQ, :]).reshape(4, 128, D)
        bqk_c = np.concatenate([bqkv[qcols], bqkv[kcols]]).reshape(1, 2 * GQ)
        bv_c = bqkv[vcols].reshape(1, GQ)
        cosT, sin_rot = tabs[b]
        in_maps.append({
            "xt": xts[b], "wqk": wqk_c, "wv": wv_c, "wo": wo_c,
            "bqk": np.ascontiguousarray(bqk_c), "bv": np.ascontiguousarray(bv_c),
            "cost": cosT, "sinrt": sin_rot, "maskd": mask,
        })

    nc = _get_nc()
    res = run_bass_kernel_spmd(nc, in_maps, core_ids=list(range(NCORES)))

    out = np.empty((B, S, D), dtype=np.float32)
    for b in range(B):
        acc = res.results[b * HG]["outt"].reshape(D, S).astype(np.float32).copy()
        for hg in range(1, HG):
            acc += res.results[b * HG + hg]["outt"].reshape(D, S)
        out[b] = acc.T + bo[None, :]
    return out
